# revision 1
# baseline (speedup 1.0000x reference)
"""Trainium2 Bass kernel for nn_MultiHeadAttention_79534204387726.

Reference computation (B=4, S=1024, E=1024, H=16, dh=64):
    q/k/v = proj(x) = x @ Wq_w.T + Wq_b       (same Wq applied to q, k, v)
    scores = q @ k.T / 8 per head; attn = softmax(scores)
    out = (attn @ v).concat_heads @ Wo_w.T + Wo_b

Sharding (8 cores): core c -> batch b = c//2, head-group g = c%2 (8 heads,
512 features). Each core computes its head-group's attention output C_g
[S, 512] and the PARTIAL output projection C_g @ Wo[:, g-half].T -> [S, E].
The host sums the two partials per batch (the "all-reduce after Wo" done
host-side) and adds a folded bias.

Math simplifications (exact):
  - K-bias is softmax-invariant (constant shift along the key axis) -> dropped.
  - V-bias passes through softmax unchanged (attn rows sum to 1), so its
    contribution is (Wo_w @ Wq_b); folded into the host-side bias with Wo_b.
  - Softmax computed without max-subtraction: scores are tightly bounded
    (|s| < ~3), exp is safe in fp32. The denominator is built by folding the
    8 key-tiles of exp(scores^T) on DVE (fp16 2x mode) and finishing with a
    gpsimd partition all-reduce, which also broadcasts 1/l to all partitions.

Layouts keep everything transposed so softmax's key-axis reduction lands on
the matmul contraction (partition) axis and no on-chip transposes are needed:
  QT/KT [j, s] -> scoresT [k, q] -> exp -> PT -> AV gives OT=C^T [d, q]
  -> out-proj uses C^T tiles as stationary operands -> out [s, o] natural.

On-chip dtypes: fp16 matmul inputs (full PE rate like bf16 but 4x the
mantissa; fp32 matmul is half-rate), fp32 PSUM accumulation everywhere,
fp32 output. Heads are processed in pairs living at partitions 0-63 /
64-127 of one j-tile: score matmuls (K=dh=64) issue back-to-back on
disjoint PE row-groups and overlap, and the AV matmuls are col-tiled
(tile_position 0/64) so the pair shares one 128-partition psum tile.
"""

import numpy as np
import ml_dtypes

B, S, E, H = 4, 1024, 1024, 16
NCORES = 8
EH = E // 2        # 512 features per head-group
NHG = H // 2       # 8 heads per group
DH = E // H        # 64
P = 128
NE = E // P        # 8 e-tiles over full E
NJ = EH // P       # 4 j-tiles over the group's 512 features
NQ = S // 512      # 2 query/sequence chunks of 512
NST = S // P       # 8 sequence tiles of 128
BF16 = np.float16

_CACHE = {}


def _build_program(reps=1, av_mode="packed"):
    import concourse.tile as tile
    from concourse import bacc, mybir
    from contextlib import ExitStack

    f32 = mybir.dt.float32
    bf16 = mybir.dt.float16
    AF = mybir.ActivationFunctionType

    nc = bacc.Bacc(
        "TRN2",
        target_bir_lowering=False,
        debug=False,
        num_devices=NCORES,
    )

    xq_t = nc.dram_tensor("xq_t", [E, S], bf16, kind="ExternalInput")
    xk_t = nc.dram_tensor("xk_t", [E, S], bf16, kind="ExternalInput")
    xv_t = nc.dram_tensor("xv_t", [E, S], bf16, kind="ExternalInput")
    wq_t = nc.dram_tensor("wq_t", [E, EH], bf16, kind="ExternalInput")
    wo_t = nc.dram_tensor("wo_t", [EH, E], bf16, kind="ExternalInput")
    bq = nc.dram_tensor("bq", [P, NJ], f32, kind="ExternalInput")
    out_d = nc.dram_tensor("out_partial", [S, E], f32, kind="ExternalOutput")

    with tile.TileContext(nc) as tc, ExitStack() as ctx:
        const = ctx.enter_context(tc.tile_pool(name="const", bufs=1))
        pt_pool = ctx.enter_context(tc.tile_pool(name="pt", bufs=4))
        fold_pool = ctx.enter_context(tc.tile_pool(name="fold", bufs=2))
        rl_pool = ctx.enter_context(tc.tile_pool(name="rl", bufs=2))
        outp = ctx.enter_context(tc.tile_pool(name="outp", bufs=6))
        ps_pool = ctx.enter_context(tc.tile_pool(name="ps", bufs=2, space="PSUM"))
        ps_s = ctx.enter_context(tc.tile_pool(name="ps_s", bufs=4, space="PSUM"))
        ps_o = ctx.enter_context(tc.tile_pool(name="ps_o", bufs=2, space="PSUM"))

        # ---- resident SBUF tensors (separate tiles per j/s-tile so the
        # scheduler's dependencies stay fine-grained) ----
        wq_sb = [const.tile([P, EH], bf16, tag=f"wq{t}", name=f"wq{t}")
                 for t in range(NE)]                     # per e-tile
        wo_sb = const.tile([P, NJ, E], bf16)             # [p, e4-tile, o]
        bq_sb = const.tile([P, NJ], f32)
        xq_sb = [const.tile([P, S], bf16, tag=f"xq{t}", name=f"xq{t}") for t in range(NE)]
        xk_sb = [const.tile([P, S], bf16, tag=f"xk{t}", name=f"xk{t}") for t in range(NE)]
        xv_sb = [const.tile([P, S], bf16, tag=f"xv{t}", name=f"xv{t}") for t in range(NE)]
        qt_sb = [const.tile([P, S], bf16, tag=f"qt{j}", name=f"qt{j}") for j in range(NJ)]
        kt_sb = [const.tile([P, S], bf16, tag=f"kt{j}", name=f"kt{j}") for j in range(NJ)]
        # V tiles [s-tile][p, 8 heads x dh (+ ones col in "ones" mode)]
        vw = DH + (1 if av_mode == "ones" else 0)
        v_sb = [const.tile([P, NHG * vw], bf16, tag=f"v{st}", name=f"v{st}")
                for st in range(NST)]
        c_sb = [const.tile([P, S], bf16, tag=f"c{j}", name=f"c{j}") for j in range(NJ)]

        nc.sync.dma_start(out=bq_sb[:, :], in_=bq[:, :])
        for t in range(NE):
            nc.sync.dma_start(out=wq_sb[t], in_=wq_t[t * P:(t + 1) * P, :])
            nc.sync.dma_start(out=xk_sb[t], in_=xk_t[t * P:(t + 1) * P, :])
        for t in range(NE):
            nc.sync.dma_start(out=xq_sb[t], in_=xq_t[t * P:(t + 1) * P, :])
        for t in range(NE):
            nc.sync.dma_start(out=xv_sb[t], in_=xv_t[t * P:(t + 1) * P, :])
        for t in range(NJ):
            nc.sync.dma_start(out=wo_sb[:, t, :], in_=wo_t[t * P:(t + 1) * P, :])

        def body():
            if av_mode == "ones":
                for st in range(NST):
                    vh = v_sb[st].rearrange("p (h c) -> p h c", c=DH + 1)
                    nc.vector.memset(vh[:, :, DH], 1.0)

            def proj_qk(jt, x_tiles, dst, bias):
                # dst[j, s] = Wq-tile.T @ x^T, j-tile jt
                for qc in range(NQ):
                    ps = ps_pool.tile([P, 512], f32, tag="ps")
                    for t in range(NE):
                        nc.tensor.matmul(
                            ps,
                            lhsT=wq_sb[t][:, jt * P:(jt + 1) * P],
                            rhs=x_tiles[t][:, qc * 512:(qc + 1) * 512],
                            start=(t == 0),
                            stop=(t == NE - 1),
                        )
                    d = dst[:, qc * 512:(qc + 1) * 512]
                    if bias is not None:
                        nc.vector.tensor_scalar_add(d, ps, bias)
                    else:
                        nc.vector.tensor_copy(d, ps)

            def proj_v(st):
                # V[s-tile, :] with ones col; strided single copy per s-tile
                ps = ps_pool.tile([P, 512], f32, tag="ps")
                for t in range(NE):
                    nc.tensor.matmul(
                        ps,
                        lhsT=xv_sb[t][:, st * P:(st + 1) * P],
                        rhs=wq_sb[t],
                        start=(t == 0),
                        stop=(t == NE - 1),
                    )
                if av_mode == "ones":
                    vh = v_sb[st].rearrange("p (h c) -> p h c", c=DH + 1)
                    nc.vector.tensor_copy(
                        vh[:, :, 0:DH], ps.rearrange("p (h d) -> p h d", d=DH))
                else:
                    nc.vector.tensor_copy(v_sb[st], ps)

            def scores_exp(jt, pt_pair):
                # score^T tiles for the head pair at j-tile jt; the two
                # heads' lhsT live at base partitions 0/64 -> adjacent MMs
                # run on disjoint PE row groups concurrently
                for kt in range(NE):
                    for qc in range(NQ):
                        pss = []
                        for hh in range(2):
                            bp = 64 * hh
                            ps = ps_s.tile([P, 512], f32, tag="ps_s")
                            pss.append(ps)
                            nc.tensor.matmul(
                                ps,
                                lhsT=kt_sb[jt][bp:bp + DH, kt * P:(kt + 1) * P],
                                rhs=qt_sb[jt][bp:bp + DH, qc * 512:(qc + 1) * 512],
                                start=True, stop=True,
                            )
                        for hh in range(2):
                            nc.scalar.activation(
                                out=pt_pair[hh][:, kt, qc * 512:(qc + 1) * 512],
                                in_=pss[hh],
                                func=AF.Exp, scale=0.125,
                            )

            def denom_qc(jt, hh, pt):
                # per-q-chunk denominator (used for the last pair): lets the
                # qc0 half of c finish early so out-proj st<4 groups unblock
                import concourse.bass_isa as bass_isa
                bp = 64 * hh
                rls = []
                for qc in range(NQ):
                    sl = slice(qc * 512, (qc + 1) * 512)
                    f = [fold_pool.tile([P, 512], bf16, tag=f"fold{i}",
                                        name=f"fq{i}") for i in range(4)]
                    for i in range(4):
                        nc.vector.tensor_add(
                            f[i], pt[:, 2 * i, sl], pt[:, 2 * i + 1, sl])
                    nc.vector.tensor_add(f[0], f[0], f[1])
                    nc.vector.tensor_add(f[2], f[2], f[3])
                    nc.vector.tensor_add(f[0], f[0], f[2])
                    rl = rl_pool.tile([P, 512], f32, tag=f"rq{hh}{qc}",
                                      name=f"rq{hh}{qc}", bufs=1)
                    nc.gpsimd.partition_all_reduce(
                        rl, f[0], channels=P, reduce_op=bass_isa.ReduceOp.add
                    )
                    nc.vector.reciprocal(rl[bp:bp + DH, :], rl[bp:bp + DH, :])
                    rls.append(rl)
                return rls

            def denom(jt, hh, pt):
                # softmax denominator for head h = 2*jt + hh: fold the 8
                # key-tiles of exp(scores^T) on DVE (fp16 4x mode), then a
                # gpsimd partition all-reduce gives l broadcast to all
                # partitions; reciprocal in place on this head's 64 rows
                import concourse.bass_isa as bass_isa
                bp = 64 * hh
                f = [fold_pool.tile([P, S], bf16, tag=f"fold{i}", name=f"fold{i}")
                     for i in range(4)]
                for i in range(4):
                    nc.vector.tensor_add(f[i], pt[:, 2 * i, :], pt[:, 2 * i + 1, :])
                nc.vector.tensor_add(f[0], f[0], f[1])
                nc.vector.tensor_add(f[2], f[2], f[3])
                nc.vector.tensor_add(f[0], f[0], f[2])
                rl = rl_pool.tile([P, S], f32, tag=f"rl{hh}", name=f"rl{hh}",
                                  bufs=1)
                nc.gpsimd.partition_all_reduce(
                    rl, f[0], channels=P, reduce_op=bass_isa.ReduceOp.add
                )
                nc.vector.reciprocal(rl[bp:bp + DH, :], rl[bp:bp + DH, :])
                return rl

            def av_ones(jt, pts):
                # M=65 AV with ones column: denominator lands in psum row DH
                for hh in range(2):
                    h = 2 * jt + hh
                    bp = 64 * hh
                    for qc in range(NQ):
                        po = ps_o.tile([P, 512], f32, tag="ps_o")
                        for kt in range(NE):
                            nc.tensor.matmul(
                                po[0:DH + 1, :],
                                lhsT=v_sb[kt][:, h * (DH + 1):(h + 1) * (DH + 1)],
                                rhs=pts[hh][:, kt, qc * 512:(qc + 1) * 512],
                                start=(kt == 0),
                                stop=(kt == NE - 1),
                            )
                        rden = rl_pool.tile([1, 512], f32, tag="rden", name="rden")
                        nc.vector.reciprocal(rden, po[DH:DH + 1, :])
                        rb = rl_pool.tile([DH, 512], f32, tag="rb", name="rb")
                        nc.gpsimd.partition_broadcast(rb, rden, channels=DH)
                        nc.vector.tensor_mul(
                            c_sb[jt][bp:bp + DH, qc * 512:(qc + 1) * 512],
                            po[0:DH, :], rb,
                        )

            def av_last(jt, pts):
                import concourse.bass_isa as bass_isa
                for qc in range(NQ):
                    sl = slice(qc * 512, (qc + 1) * 512)
                    rls = []
                    for hh in range(2):
                        bp = 64 * hh
                        f = [fold_pool.tile([P, 512], bf16, tag=f"fold{i}",
                                            name=f"fl{i}") for i in range(4)]
                        for i in range(4):
                            nc.vector.tensor_add(
                                f[i], pts[hh][:, 2 * i, sl],
                                pts[hh][:, 2 * i + 1, sl])
                        nc.vector.tensor_add(f[0], f[0], f[1])
                        nc.vector.tensor_add(f[2], f[2], f[3])
                        nc.vector.tensor_add(f[0], f[0], f[2])
                        rl = rl_pool.tile([P, 512], f32, tag=f"rq{hh}{qc}",
                                          name=f"rq{hh}{qc}", bufs=1)
                        nc.gpsimd.partition_all_reduce(
                            rl, f[0], channels=P,
                            reduce_op=bass_isa.ReduceOp.add)
                        nc.vector.reciprocal(rl[bp:bp + DH, :], rl[bp:bp + DH, :])
                        rls.append(rl)
                    po = ps_o.tile([P, 512], f32, tag="ps_o")
                    for kt in range(NE):
                        for hh in range(2):
                            h = 2 * jt + hh
                            bp = 64 * hh
                            nc.tensor.matmul(
                                po[bp:bp + DH, :],
                                lhsT=v_sb[kt][:, h * DH:(h + 1) * DH],
                                rhs=pts[hh][:, kt, sl],
                                start=(kt == 0),
                                stop=(kt == NE - 1),
                                tile_position=(0, bp),
                            )
                    for hh in range(2):
                        bp = 64 * hh
                        nc.vector.tensor_mul(
                            c_sb[jt][bp:bp + DH, sl],
                            po[bp:bp + DH, :], rls[hh][bp:bp + DH, :],
                        )

            def av_pair(jt, pts, rls):
                # col-tiled AV: head A on PE columns 0-63 -> psum rows 0-63,
                # head B on columns 64-127 -> psum rows 64-127
                for qc in range(NQ):
                    po = ps_o.tile([P, 512], f32, tag="ps_o")
                    for kt in range(NE):
                        for hh in range(2):
                            h = 2 * jt + hh
                            bp = 64 * hh
                            nc.tensor.matmul(
                                po[bp:bp + DH, :],
                                lhsT=v_sb[kt][:, (2 * jt + hh) * DH:(2 * jt + hh + 1) * DH],
                                rhs=pts[hh][:, kt, qc * 512:(qc + 1) * 512],
                                start=(kt == 0),
                                stop=(kt == NE - 1),
                                tile_position=(0, bp),
                            )
                    for hh in range(2):
                        bp = 64 * hh
                        r = rls[hh]
                        rsl = (r[qc][bp:bp + DH, :] if isinstance(r, list)
                               else r[bp:bp + DH, qc * 512:(qc + 1) * 512])
                        nc.vector.tensor_mul(
                            c_sb[jt][bp:bp + DH, qc * 512:(qc + 1) * 512],
                            po[bp:bp + DH, :], rsl,
                        )

            def out_proj(st, oc):
                ps = ps_pool.tile([P, 512], f32, tag="ps")
                for et in range(NJ):
                    nc.tensor.matmul(
                        ps,
                        lhsT=c_sb[et][:, st * P:(st + 1) * P],
                        rhs=wo_sb[:, et, oc * 512:(oc + 1) * 512],
                        start=(et == 0),
                        stop=(et == NJ - 1),
                    )
                ot = outp.tile([P, 512], f32, tag="ot")
                nc.scalar.copy(ot, ps)
                nc.sync.dma_start(
                    out=out_d[st * P:(st + 1) * P, oc * 512:(oc + 1) * 512],
                    in_=ot,
                )

            # ---- emission order: interleave so V-proj / next j-tile's
            # projections (PE work) can fill the ACT-bound exp stretches ----
            # pipeline: pair 0's scores first, then all of V, then each
            # subsequent pair's scores followed by the previous pair's AV --
            # so pt/rl pool slots (bufs=2) recycle without stalling the flow
            pt_pairs = []
            rl_pairs = []

            def scores_block(jt):
                proj_qk(jt, xk_sb, kt_sb[jt], None)
                proj_qk(jt, xq_sb, qt_sb[jt], bq_sb[:, jt:jt + 1])
                pair = [pt_pool.tile([P, NE, S], bf16, tag="pt", name=f"pt{hh}")
                        for hh in range(2)]
                pt_pairs.append(pair)
                scores_exp(jt, pair)
                if av_mode == "ones":
                    rl_pairs.append(None)
                elif jt == NJ - 1:
                    rl_pairs.append(None)  # handled inside av_last
                else:
                    rl_pairs.append([denom(jt, hh, pair[hh]) for hh in range(2)])

            scores_block(0)
            for st in range(NST):
                proj_v(st)
            def do_av(jt):
                if av_mode == "ones":
                    av_ones(jt, pt_pairs[jt])
                elif jt == NJ - 1:
                    av_last(jt, pt_pairs[jt])
                else:
                    av_pair(jt, pt_pairs[jt], rl_pairs[jt])

            for jt in range(1, NJ):
                scores_block(jt)
                do_av(jt - 1)
            do_av(NJ - 1)
            for st in range(NST):
                for oc in range(NQ):
                    out_proj(st, oc)

        for _ in range(reps):
            body()

    nc.finalize()
    return nc


def _get_nc(reps=1, av_mode="packed"):
    key = ("nc", reps, av_mode)
    if key not in _CACHE:
        _CACHE[key] = _build_program(reps, av_mode)
    return _CACHE[key]


def make_in_maps(queries, keys, values, Wq_w, Wq_b, Wo_w, Wo_b):
    in_maps = []
    for c in range(NCORES):
        b, g = c // 2, c % 2
        js = slice(g * EH, (g + 1) * EH)
        in_maps.append({
            "xq_t": np.ascontiguousarray(queries[b].T).astype(BF16),
            "xk_t": np.ascontiguousarray(keys[b].T).astype(BF16),
            "xv_t": np.ascontiguousarray(values[b].T).astype(BF16),
            "wq_t": np.ascontiguousarray(Wq_w[js, :].T).astype(BF16),
            "wo_t": np.ascontiguousarray(Wo_w[:, js].T).astype(BF16),
            "bq": np.ascontiguousarray(Wq_b[js].reshape(NJ, P).T),
        })
    return in_maps


def assemble_output(results, Wq_b, Wo_w, Wo_b):
    # host-side unshard: sum the two head-group partials per batch, add the
    # folded bias (Wo_b + V-bias routed through Wo since attn rows sum to 1)
    bias_total = (Wo_w @ Wq_b + Wo_b).astype(np.float32)
    out = np.empty((B, S, E), np.float32)
    for b in range(B):
        out[b] = results[2 * b]["out_partial"] + results[2 * b + 1]["out_partial"]
    out += bias_total
    return out


def kernel(queries, keys, values, Wq_w, Wq_b, Wo_w, Wo_b, num_heads):
    from concourse.bass_utils import run_bass_kernel_spmd

    queries = np.asarray(queries, np.float32)
    keys = np.asarray(keys, np.float32)
    values = np.asarray(values, np.float32)
    Wq_w = np.asarray(Wq_w, np.float32)
    Wq_b = np.asarray(Wq_b, np.float32)
    Wo_w = np.asarray(Wo_w, np.float32)
    Wo_b = np.asarray(Wo_b, np.float32)
    assert int(num_heads) == H

    nc = _get_nc()
    in_maps = make_in_maps(queries, keys, values, Wq_w, Wq_b, Wo_w, Wo_b)
    res = run_bass_kernel_spmd(nc, in_maps, core_ids=list(range(NCORES)))
    _CACHE["last_results"] = res
    return assemble_output(res.results, Wq_b, Wo_w, Wo_b)



# revision 39
# speedup vs baseline: 12.7661x; 12.7661x over previous
"""Trainium2 Bass kernel for nn_MultiHeadAttention_79534204387726.

Reference computation (B=4, S=1024, E=1024, H=16, dh=64):
    q/k/v = proj(x) = x @ Wq_w.T + Wq_b       (same Wq applied to q, k, v)
    scores = q @ k.T / 8 per head; attn = softmax(scores)
    out = (attn @ v).concat_heads @ Wo_w.T + Wo_b

Sharding (8 cores): core c -> batch b = c//2, head-group g = c%2 (8 heads,
512 features). Each core computes its head-group's attention output C_g
[S, 512] and the PARTIAL output projection C_g @ Wo[:, g-half].T -> [S, E].
The host sums the two partials per batch (the "all-reduce after Wo" done
host-side) and adds a folded bias.

Math simplifications (exact):
  - K-bias is softmax-invariant (constant shift along the key axis) -> dropped.
  - V-bias passes through softmax unchanged (attn rows sum to 1), so its
    contribution is (Wo_w @ Wq_b); folded into the host-side bias with Wo_b.
  - Softmax computed without max-subtraction: scores are tightly bounded
    (|s| < ~3), exp is safe in fp32.

fp8 DoubleRow acceleration (PE runs fp8e4 at 0.5 cycles/row with
perf_mode=DoubleRow, contracting 2x128 rows per instruction):
  - Q/K projections: X single-fp8 (x2 scale), Wq double-fp8 (hi + residual,
    x16 scale -- the shared-W residual cleans BOTH q and k paths for one
    extra pass). 8 DR matmuls per psum tile, each contracting 256 rows.
  - Scores: Q stored single-fp8 (bias added, written to both slabs), K
    stored double-fp8 (hi + residual). One DR matmul per (kt, qc, head)
    contracts the 64 head dims via [64, 2, *] slabs: slab0 = K_hi x Q,
    slab1 = K_lo x Q. 2x fewer PE cycles than bf16. Stored q,k carry a
    32x scale (kept well under the TRN e4m3 +-240 saturation point, which
    turns overflows into inf); the exp scale absorbs 1/1024.
  - V path / AV / out-projection stay fp16: their quantization error would
    hit the output linearly (no softmax attenuation). Measured end-to-end
    rel err ~1.46e-2 against the f32 reference (gate 2e-2).

Softmax denominator via the ones-column trick ("ones" AV mode): V tiles
carry a 65th column of 1.0, so the AV matmul's psum row 64 accumulates
l = sum_k exp(s). Reciprocal [1,512] on DVE + gpsimd partition_broadcast
+ fused multiply on the C writeback. No fold chains / partition
all-reduces (frees ~30us of DVE for the fp8 stores and out copies).

Layouts keep everything transposed so softmax's key-axis reduction lands on
the matmul contraction (partition) axis and no on-chip transposes are needed:
  QT/KT [j, s] -> scoresT [k, q] -> exp -> PT -> AV gives OT=C^T [d, q]
  -> out-proj uses C^T tiles as stationary operands -> out [s, o] natural.

Scores psum tiles are [128, 1024] (2 psum banks, one per qc chunk) so each
exp activation covers 1024 columns, amortizing ACT per-instruction overhead;
exp is the dominant ACT load (~64 instructions, ~67us: the pacing conveyor).

Scheduling: engines execute their streams in emission order with a shallow
wait-queue, so every unit is emitted only after its inputs' producers and
near their data's DMA arrival. Each round jt = 16 scores units (2 DR matmuls
gated on a psum slot the exp frees); rounds 2-3 ladder their own AV matmuls
two exps behind the conveyor; rounds 0-1 defer AV to whole units in rounds
1-2 (their V operands stream in during round 0). The out-projection is split:
c0/c1 partials (visit1) run inside rounds 3+ into fp16 sbuf, and after the
last AV only c2/c3 + an identity-matmul re-add of the partial + copy + DMA
remain. Input DMA is ordered by first use across the SP and ACT hardware DGE
queues; a dozen DoubleRow warm-up matmuls pin the PE p-state ramp during the
prologue. Output partials are fp16 to halve the closing DMA wire time.
"""

import numpy as np
import ml_dtypes

B, S, E, H = 4, 1024, 1024, 16
NCORES = 8
EH = E // 2        # 512 features per head-group
NHG = H // 2       # 8 heads per group
DH = E // H        # 64
P = 128
NE = E // P        # 8 e-tiles over full E
NE2 = NE // 2      # 4 DR pair-tiles
NJ = EH // P       # 4 j-tiles over the group's 512 features
NQ = S // 512      # 2 query/sequence chunks of 512
NST = S // P       # 8 sequence tiles of 128
F16 = np.float16
FP8 = ml_dtypes.float8_e4m3
SX = 2.0           # X scale into fp8 (SQ=SX*SW=32: keeps stored q,k
                   # well under the TRN e4m3 +-240 saturation range)
X_DOUBLE = False   # X double-fp8 (hi+lo) in Q/K proj
W_DOUBLE = True    # Wq double-fp8: one residual fixes BOTH q and k paths
SW = 16.0          # W scale into fp8
SQ = SX * SW       # stored q/k scale (64)
VW = DH + 1        # V tile width per head (ones column appended)
ONES_VAL = 1.0

_CACHE = {}


def _build_program(reps=1, debug=False):
    import concourse.tile as tile
    from concourse import bacc, mybir
    from contextlib import ExitStack

    f32 = mybir.dt.float32
    f16 = mybir.dt.float16
    fp8 = mybir.dt.float8e4
    DR = mybir.MatmulPerfMode.DoubleRow
    AF = mybir.ActivationFunctionType

    nc = bacc.Bacc(
        "TRN2",
        target_bir_lowering=False,
        debug=False,
        num_devices=NCORES,
    )

    NHL = 2 if X_DOUBLE else 1
    xq8_d = nc.dram_tensor("xq8", [P, NE2, NHL, 2, S], fp8, kind="ExternalInput")
    xk8_d = nc.dram_tensor("xk8", [P, NE2, NHL, 2, S], fp8, kind="ExternalInput")
    NWL = 2 if W_DOUBLE else 1
    wq8_d = nc.dram_tensor("wq8", [P, NE2, NWL, 2, EH], fp8, kind="ExternalInput")
    xv_d = nc.dram_tensor("xv16", [P, NE, S], f16, kind="ExternalInput")
    wq16_d = nc.dram_tensor("wq16", [P, NE, EH], f16, kind="ExternalInput")
    wo_d = nc.dram_tensor("wo16", [P, NJ, E], f16, kind="ExternalInput")
    bq = nc.dram_tensor("bq", [P, NJ], f32, kind="ExternalInput")   # 64*Wq_b
    ident_d = nc.dram_tensor("ident", [P, P], f16, kind="ExternalInput")
    if debug:
        dbg_qt = nc.dram_tensor("dbg_qt", [P, 2, S], mybir.dt.float8e4, kind="ExternalOutput")
        dbg_kt = nc.dram_tensor("dbg_kt", [P, 2, S], mybir.dt.float8e4, kind="ExternalOutput")
        dbg_pt = nc.dram_tensor("dbg_pt", [P, NE, S], f16, kind="ExternalOutput")
        dbg_c = nc.dram_tensor("dbg_c", [P, 512], f16, kind="ExternalOutput")
        dbg_v = nc.dram_tensor("dbg_v", [P, NHG * VW], f16, kind="ExternalOutput")
    out_d = nc.dram_tensor("out_partial", [S, E], f16, kind="ExternalOutput")

    with tile.TileContext(nc) as tc, ExitStack() as ctx:
        const = ctx.enter_context(tc.tile_pool(name="const", bufs=1))
        pt_pool = ctx.enter_context(tc.tile_pool(name="pt", bufs=4))
        rl_pool = ctx.enter_context(tc.tile_pool(name="rl", bufs=2))
        outp = ctx.enter_context(tc.tile_pool(name="outp", bufs=3))
        ps_pool = ctx.enter_context(tc.tile_pool(name="ps", bufs=2, space="PSUM"))
        ps_s = ctx.enter_context(tc.tile_pool(name="ps_s", bufs=2, space="PSUM"))
        ps_o = ctx.enter_context(tc.tile_pool(name="ps_o", bufs=2, space="PSUM"))

        # ---- resident SBUF tensors ----
        xq8t = const.tile([P, NE2, NHL, 2, S], fp8, tag="xq8", name="xq8")
        xk8t = const.tile([P, NE2, NHL, 2, S], fp8, tag="xk8", name="xk8")
        xq8h = [xq8t[:, t, 0, :, :] for t in range(NE2)]
        xk8h = [xk8t[:, t, 0, :, :] for t in range(NE2)]
        xq8l = [xq8t[:, t, 1, :, :] for t in range(NE2)] if X_DOUBLE else None
        xk8l = [xk8t[:, t, 1, :, :] for t in range(NE2)] if X_DOUBLE else None
        wq8t = const.tile([P, NE2, NWL, 2, EH], fp8, tag="wq8", name="wq8")
        wq8_sb = [wq8t[:, t, 0, :, :] for t in range(NE2)]
        wq8_lo = ([wq8t[:, t, 1, :, :] for t in range(NE2)] if W_DOUBLE else None)
        wq16t = const.tile([P, NE, EH], f16, tag="wq16", name="wq16")
        wq_sb = [wq16t[:, t, :] for t in range(NE)]
        xv16t = const.tile([P, NE, S], f16, tag="xv16", name="xv16")
        xv_sb = [xv16t[:, t, :] for t in range(NE)]
        wo_sb = const.tile([P, NJ, E], f16)
        bq_sb = const.tile([P, NJ], f32)
        qt8 = [const.tile([P, 2, S], fp8, tag=f"qt{j}", name=f"qt{j}")
               for j in range(NJ)]
        kt8 = [const.tile([P, 2, S], fp8, tag=f"kt{j}", name=f"kt{j}")
               for j in range(NJ)]
        v_sb = [const.tile([P, NHG * VW], f16, tag=f"v{st}", name=f"v{st}")
                for st in range(NST)]
        c_sb = [[const.tile([P, 512], f16, tag=f"c{j}_{qc}", name=f"c{j}_{qc}")
                 for qc in range(NQ)] for j in range(NJ)]
        ident_sb = const.tile([P, P], f16, tag="ident", name="ident")
        part_sb = [[const.tile([P, 512], f16, tag=f"pp{s}_{o}", name=f"pp{s}_{o}")
                    for o in range(NQ)] for s in range(NST)]

        # DMA in first-use order: K-proj qc0 inputs, then Q-proj, then the
        # K qc1 halves, then the fp16 V-path tensors, then Wo
        # paired queue order so the serial DMA wire serves every QK-critical
        # transfer before the (big, later-needed) fp16 V-path tensors
        nc.sync.dma_start(out=wq8t, in_=wq8_d[:, :, :, :, :])
        nc.scalar.dma_start(out=bq_sb[:, :], in_=bq[:, :])
        for t in range(NE2):
            (nc.sync if t % 2 else nc.scalar).dma_start(
                out=xq8t[:, t, :, :, :], in_=xq8_d[:, t, :, :, :])
        for t in range(NE2):
            (nc.sync if t % 2 else nc.scalar).dma_start(
                out=xk8t[:, t, :, :, :], in_=xk8_d[:, t, :, :, :])
        nc.sync.dma_start(out=wq16t, in_=wq16_d[:, :, :])
        nc.scalar.dma_start(out=xv16t, in_=xv_d[:, :, :])
        nc.scalar.dma_start(out=wo_sb, in_=wo_d[:, :, :])
        nc.sync.dma_start(out=ident_sb, in_=ident_d[:, :])

        warm = const.tile([1, 1], f32, tag="warm", name="warm")

        def body():
            nc.scalar.activation(out=warm, in_=bq_sb[0:1, 0:1],
                                 func=AF.Exp, scale=0.0)
            for _ in range(12):
                dps = ps_pool.tile([P, 512], f32, tag="ps", name="dummy")
                nc.tensor.matmul(
                    dps, lhsT=wq8t[:, 0, 0, :, 0:P], rhs=wq8t[:, 0, 0, :, 0:512],
                    start=True, stop=True, perf_mode=DR)
            for st in range(NST):
                vh = v_sb[st].rearrange("p (h c) -> p h c", c=VW)
                nc.vector.memset(vh[:, :, DH], ONES_VAL)

            def proj_dr(jt, xh, xl, store, qc):
                # psum = 32*(x @ Wq_g.T) via DoubleRow fp8 matmuls
                ps = ps_pool.tile([P, 512], f32, tag="ps")
                sl = slice(qc * 512, (qc + 1) * 512)
                passes = [(xh, wq8_sb)]
                if X_DOUBLE:
                    passes.append((xl, wq8_sb))
                if W_DOUBLE:
                    passes.append((xh, wq8_lo))
                for pi, (xs, ws) in enumerate(passes):
                    for t2 in range(NE2):
                        nc.tensor.matmul(
                            ps,
                            lhsT=ws[t2][:, :, jt * P:(jt + 1) * P],
                            rhs=xs[t2][:, :, sl],
                            start=(pi == 0 and t2 == 0),
                            stop=(pi == len(passes) - 1 and t2 == NE2 - 1),
                            perf_mode=DR,
                        )
                store(ps, sl)

            def store_k(jt):
                def store(ps, sl):
                    nc.vector.tensor_copy(kt8[jt][:, 0, sl], ps)
                    nc.vector.tensor_sub(kt8[jt][:, 1, sl], ps, kt8[jt][:, 0, sl])
                return store

            def store_q(jt):
                def store(ps, sl):
                    nc.vector.tensor_scalar_add(
                        qt8[jt][:, 0, sl], ps, bq_sb[:, jt:jt + 1])
                    nc.gpsimd.tensor_copy(qt8[jt][:, 1, sl], qt8[jt][:, 0, sl])
                return store

            v_ps = {}

            def proj_v_half(st, half):
                # V[s-tile, :]; two half-units so fillers stay fine-grained
                if half == 0:
                    v_ps[st] = ps_pool.tile([P, 512], f32, tag="ps",
                                            name=f"vps{st}")
                ps = v_ps[st]
                for t in range(4 * half, 4 * half + 4):
                    nc.tensor.matmul(
                        ps,
                        lhsT=xv_sb[t][:, st * P:(st + 1) * P],
                        rhs=wq_sb[t],
                        start=(t == 0),
                        stop=(t == NE - 1),
                    )
                if half == 1:
                    vh = v_sb[st].rearrange("p (h c) -> p h c", c=VW)
                    nc.vector.tensor_copy(
                        vh[:, :, 0:DH], ps.rearrange("p (h d) -> p h d", d=DH))
                    del v_ps[st]

            def sc_unit(jt, pt_pair, hh, kt):
                # scoresT tile: one DR matmul per (kt, qc, head). K double
                # (hi slab0 / lo slab1), Q duplicated across slabs. Both
                # carry a 64x scale -> exp scale 0.125/4096.
                bp = 64 * hh
                ps = ps_s.tile([P, 1024], f32, tag="ps_s")
                for qc in range(NQ):
                    nc.tensor.matmul(
                        ps[:, qc * 512:(qc + 1) * 512],
                        lhsT=kt8[jt][bp:bp + DH, :, kt * P:(kt + 1) * P],
                        rhs=qt8[jt][bp:bp + DH, :, qc * 512:(qc + 1) * 512],
                        start=True, stop=True,
                        perf_mode=DR,
                    )
                nc.scalar.activation(
                    out=pt_pair[hh][:, kt, :],
                    in_=ps,
                    func=AF.Exp, scale=0.125 / (SQ * SQ),
                )

            def av_unit(jt, pts, qc, hh):
                # M=65 AV with ones column: denominator lands in psum row 64
                h = 2 * jt + hh
                bp = 64 * hh
                po = ps_o.tile([P, 512], f32, tag="ps_o")
                for kt in range(NE):
                    nc.tensor.matmul(
                        po[0:VW, :],
                        lhsT=v_sb[kt][:, h * VW:(h + 1) * VW],
                        rhs=pts[hh][:, kt, qc * 512:(qc + 1) * 512],
                        start=(kt == 0),
                        stop=(kt == NE - 1),
                    )
                rden = rl_pool.tile([1, 512], f32, tag="rden", name="rden")
                nc.vector.reciprocal(rden, po[DH:DH + 1, :])
                rb = rl_pool.tile([DH, 512], f32, tag="rb", name="rb")
                nc.gpsimd.partition_broadcast(rb, rden, channels=DH)
                nc.vector.tensor_mul(
                    c_sb[jt][qc][bp:bp + DH, :],
                    po[0:DH, :], rb,
                )

            def av_unit_w(jt, pts, qc, hh):
                # whole-unit AV (rounds 0-1, deferred until V is emitted)
                h = 2 * jt + hh
                bp = 64 * hh
                po = ps_pool.tile([P, 512], f32, tag="ps", name=f"av{jt}_{qc}{hh}")
                for kt in range(NE):
                    nc.tensor.matmul(
                        po[0:VW, :],
                        lhsT=v_sb[kt][:, h * VW:(h + 1) * VW],
                        rhs=pts[hh][:, kt, qc * 512:(qc + 1) * 512],
                        start=(kt == 0),
                        stop=(kt == NE - 1),
                    )
                rden = rl_pool.tile([1, 512], f32, tag="rden", name="rden")
                nc.vector.reciprocal(rden, po[DH:DH + 1, :])
                rb = rl_pool.tile([DH, 512], f32, tag="rb", name="rb")
                nc.gpsimd.partition_broadcast(rb, rden, channels=DH)
                nc.vector.tensor_mul(
                    c_sb[jt][qc][bp:bp + DH, :], po[0:DH, :], rb)

            def out_visit1(st, oc):
                # partial over c0,c1 -> sbuf f16; emitted inside rounds 2-3
                qc, sx = st // 4, st % 4
                ps = ps_pool.tile([P, 512], f32, tag="ps", name=f"ov1_{st}_{oc}")
                for et in range(2):
                    nc.tensor.matmul(
                        ps,
                        lhsT=c_sb[et][qc][:, sx * P:(sx + 1) * P],
                        rhs=wo_sb[:, et, oc * 512:(oc + 1) * 512],
                        start=(et == 0),
                        stop=(et == 1),
                    )
                nc.vector.tensor_copy(part_sb[st][oc], ps)

            out_tiles = {}

            def out_visit2(st, oc, i):
                # c2,c3 contributions + identity-matmul re-add of the partial
                qc, sx = st // 4, st % 4
                pool = ps_pool if i % 2 == 0 else ps_o
                ps = pool.tile([P, 512], f32,
                               tag="ps" if i % 2 == 0 else "ps_o",
                               name=f"ov2_{st}_{oc}")
                for et in range(2, NJ):
                    nc.tensor.matmul(
                        ps,
                        lhsT=c_sb[et][qc][:, sx * P:(sx + 1) * P],
                        rhs=wo_sb[:, et, oc * 512:(oc + 1) * 512],
                        start=(et == 2),
                        stop=False,
                    )
                nc.tensor.matmul(
                    ps,
                    lhsT=ident_sb,
                    rhs=part_sb[st][oc],
                    start=False,
                    stop=True,
                )
                if oc == 0:
                    out_tiles[st] = outp.tile([P, 1024], f16, tag="ot",
                                              name=f"ot{st}")
                ot = out_tiles[st]
                dst = ot[:, oc * 512:(oc + 1) * 512]
                if i % 2 == 0:
                    nc.scalar.copy(dst, ps)
                else:
                    nc.vector.tensor_copy(dst, ps)
                if oc == 1:
                    nc.sync.dma_start(
                        out=out_d[st * P:(st + 1) * P, :], in_=ot)

            # ---- emission: per-engine streams execute in emission order.
            # Each round jt is paced by its 16 exps (ACT ~1040ns each); the
            # PE stream is the exp conveyor (2 DR matmuls per scores unit,
            # gated on a psum slot the exp frees) with the round's own AV
            # matmuls LADDERED in 2 exps behind (each AV kt-step only needs
            # exp kt, so AV accumulation tracks the conveyor and only the
            # reciprocal chain remains at the half-round boundary), plus the
            # next round's Q/K projections and a share of the V projection
            # as fillers. After the last round only the out-projection runs.
            pt_pairs = []

            def proj_units(jt):
                return [
                    lambda: proj_dr(jt, xq8h, xq8l, store_q(jt), 0),
                    lambda: proj_dr(jt, xq8h, xq8l, store_q(jt), 1),
                    lambda: proj_dr(jt, xk8h, xk8l, store_k(jt), 0),
                    lambda: proj_dr(jt, xk8h, xk8l, store_k(jt), 1),
                ]

            v_all = [(lambda st=st, h=h: proj_v_half(st, h))
                     for st in range(NST) for h in range(2)]

            def round_emit(jt):
                pair = [pt_pool.tile([P, NE, S], f16, tag="pt", name=f"pt{hh}")
                        for hh in range(2)]
                pt_pairs.append(pair)
                # rounds 0-1: no AV ladder (V still streaming in); their AV
                # runs as whole units in rounds 2/3 once every v_sb write is
                # emitted. visit1 needs c0 (round 2) and c1 (round 3).
                fill = []
                if jt == 0:
                    fill += v_all + proj_units(1)
                elif jt == 1:
                    fill += [(lambda qc=qc, hh=hh: av_unit_w(0, pt_pairs[0], qc, hh))
                             for hh in range(2) for qc in range(NQ)]
                    fill += proj_units(2)
                elif jt == 2:
                    fill += [(lambda qc=qc, hh=hh: av_unit_w(1, pt_pairs[1], qc, hh))
                             for hh in range(2) for qc in range(NQ)]
                    fill += proj_units(3)
                else:
                    fill += [(lambda s=s, o=o: out_visit1(s, o))
                             for s in range(NST) for o in range(NQ) if s < 4]
                po = {}

                def av_mm(qc, hh, kt):
                    h = 2 * jt + hh
                    if kt == 0:
                        po[(qc, hh)] = ps_o.tile(
                            [P, 512], f32, tag="ps_o", name=f"po{qc}{hh}")
                    nc.tensor.matmul(
                        po[(qc, hh)][0:VW, :],
                        lhsT=v_sb[kt][:, h * VW:(h + 1) * VW],
                        rhs=pair[hh][:, kt, qc * 512:(qc + 1) * 512],
                        start=(kt == 0),
                        stop=(kt == NE - 1),
                    )

                def av_fin(qc, hh):
                    bp = 64 * hh
                    p = po[(qc, hh)]
                    rden = rl_pool.tile([1, 512], f32, tag="rden", name="rden")
                    nc.vector.reciprocal(rden, p[DH:DH + 1, :])
                    rb = rl_pool.tile([DH, 512], f32, tag="rb", name="rb")
                    nc.gpsimd.partition_broadcast(rb, rden, channels=DH)
                    nc.vector.tensor_mul(
                        c_sb[jt][qc][bp:bp + DH, :], p[0:DH, :], rb)

                off = 2
                ladder = jt > 1
                done = 0
                pos = 0
                for hh in range(2):
                    for kt in range(NE):
                        sc_unit(jt, pair, hh, kt)
                        if ladder and kt >= off:
                            av_mm(0, hh, kt - off)
                            av_mm(1, hh, kt - off)
                        pos += 1
                        if jt == 0:
                            want = max(0, pos - 6) * len(fill) // 10
                        else:
                            want = pos * len(fill) // 16
                        while done < want:
                            fill[done]()
                            done += 1
                    if ladder:
                        for kt in range(NE - off, NE):
                            av_mm(0, hh, kt)
                            av_mm(1, hh, kt)
                        av_fin(0, hh)
                        av_fin(1, hh)

            for u in proj_units(0):
                u()
            for jt in range(NJ):
                round_emit(jt)
            for st in range(4, NST):
                for oc in range(NQ):
                    out_visit1(st, oc)
            for i, (st, oc) in enumerate(
                    (s, o) for s in range(NST) for o in range(NQ)):
                out_visit2(st, oc, i)
            if debug:
                nc.sync.dma_start(out=dbg_qt[:, :, :], in_=qt8[0])
                nc.sync.dma_start(out=dbg_kt[:, :, :], in_=kt8[0])
                nc.sync.dma_start(out=dbg_pt[:, :, :], in_=pt_pairs[0][0])
                nc.sync.dma_start(out=dbg_c[:, :], in_=c_sb[0][0])
                nc.sync.dma_start(out=dbg_v[:, :], in_=v_sb[0])

        for _ in range(reps):
            body()

    nc.finalize()
    return nc


def _get_nc(reps=1):
    key = ("nc", reps)
    if key not in _CACHE:
        _CACHE[key] = _build_program(reps)
    return _CACHE[key]


def _dr_pack(xt):
    """[E, S] f32 -> fp8 [P, NE2, hl, 2(slab), S]: row r=(2*t2+i)*128+p."""
    xs = xt.reshape(NE2, 2, P, -1).transpose(2, 0, 1, 3)   # [P, NE2, 2, S]
    hi = xs.astype(FP8)
    if not X_DOUBLE:
        return np.ascontiguousarray(hi[:, :, None, :, :])
    lo = (xs - hi.astype(np.float32)).astype(FP8)
    return np.ascontiguousarray(np.stack([hi, lo], axis=2))


def make_in_maps(queries, keys, values, Wq_w, Wq_b, Wo_w, Wo_b):
    in_maps = []
    for c in range(NCORES):
        b, g = c // 2, c % 2
        js = slice(g * EH, (g + 1) * EH)
        wq_g = Wq_w[js, :]                                  # [EH, E]
        q8p = _dr_pack(SX * queries[b].T)
        k8p = _dr_pack(SX * keys[b].T)
        w8f = (SW * wq_g.T).reshape(NE2, 2, P, EH).transpose(2, 0, 1, 3)
        w8hi = w8f.astype(FP8)
        w8lo = (w8f - w8hi.astype(np.float32)).astype(FP8)
        w8 = np.stack([w8hi, w8lo], axis=2)         # [P, NE2, 2(hl), 2, EH]
        in_maps.append({
            "xq8": q8p, "xk8": k8p,
            "wq8": np.ascontiguousarray(w8),
            "xv16": np.ascontiguousarray(
                values[b].T.reshape(NE, P, S).transpose(1, 0, 2)).astype(F16),
            "wq16": np.ascontiguousarray(
                wq_g.T.reshape(NE, P, EH).transpose(1, 0, 2)).astype(F16),
            "wo16": np.ascontiguousarray(
                Wo_w[:, js].T.reshape(NJ, P, E).transpose(1, 0, 2)).astype(F16),
            "bq": np.ascontiguousarray(
                (SQ * Wq_b[js]).reshape(NJ, P).T.astype(np.float32)),
            "ident": np.eye(P, dtype=F16),
        })
    return in_maps


def assemble_output(results, Wq_b, Wo_w, Wo_b):
    # host-side unshard: sum the two head-group partials per batch, add the
    # folded bias (Wo_b + V-bias routed through Wo since attn rows sum to 1)
    bias_total = (Wo_w @ Wq_b + Wo_b).astype(np.float32)
    out = np.empty((B, S, E), np.float32)
    for b in range(B):
        out[b] = (np.asarray(results[2 * b]["out_partial"], np.float32)
                  + np.asarray(results[2 * b + 1]["out_partial"], np.float32))
    out += bias_total
    return out


def kernel(queries, keys, values, Wq_w, Wq_b, Wo_w, Wo_b, num_heads):
    from concourse.bass_utils import run_bass_kernel_spmd

    queries = np.asarray(queries, np.float32)
    keys = np.asarray(keys, np.float32)
    values = np.asarray(values, np.float32)
    Wq_w = np.asarray(Wq_w, np.float32)
    Wq_b = np.asarray(Wq_b, np.float32)
    Wo_w = np.asarray(Wo_w, np.float32)
    Wo_b = np.asarray(Wo_b, np.float32)
    assert int(num_heads) == H

    nc = _get_nc()
    in_maps = make_in_maps(queries, keys, values, Wq_w, Wq_b, Wo_w, Wo_b)
    res = run_bass_kernel_spmd(nc, in_maps, core_ids=list(range(NCORES)))
    _CACHE["last_results"] = res
    return assemble_output(res.results, Wq_b, Wo_w, Wo_b)
